# revision 7
# baseline (speedup 1.0000x reference)
"""Trainium2 Bass kernel for nn_Attention_4063039062503.

Reference (per batch b, C=128 channels, N=4096 points):
    q = W1 @ x + b1;  k = W2 @ x + b2          # [C, N]
    s[n, m] = q[:, n] . k[:, m]                # [N, N]
    a = softmax(s, axis=m)
    out = relu(x + x @ a.T)                    # out[:, n] = x @ a[n, :]

Sharding: 8 cores, core i -> batch i//2, query half i%2 (2048 queries),
full 4096 keys local (no collectives).

Per-core plan (v2, E-stationary O-matmul):
  - exp split across engines computing IDENTICAL e^(s-25) per tile:
      ACT: exp(s - 25) -> bf16 (exact, bias-only)
      DVE: Schraudolph bits = s*(128/ln2) + BMAG -> uint16 == bf16(e^(s-25));
           the fp32->uint16 convert saturates negatives to 0 (measured), so
           no explicit clamp is needed
    (mixed-tile bias calibrated via BMAG c=5; end-to-end sim err 6.5e-3)
  - O-matmul flipped: stationary = E q-chunk [m,128], moving = [x^T | 1]
    [m,129] -> psum O'[q, 0:128] with the softmax row-sum landing free in
    col 128 (kills the DVE row-sum accumulation entirely)
  - zero-weight dummy matmuls open each O accumulation chain: hardware
    drops the first term of a start=True 129-col chain (measured), dummies
    make every real matmul start=False
  - tail per q-chunk: 1/r via reciprocal, (O*bc + x^T) on DVE, relu, DMA
    out in [q, c] orientation; host transposes back
"""
from contextlib import ExitStack

import numpy as np
import ml_dtypes

import concourse.tile as tile
from concourse import bacc, mybir
from concourse.bass_utils import run_bass_kernel_spmd

B = 4
C = 128
N = 4096            # keys per batch
NQ = 2048           # queries per core
QB = 512            # S-matmul moving block (PSUM bank)
MT = 128            # m (key) tile
N_MT = N // MT      # 32
PASS_Q = 1024       # queries per pass
N_PASS = NQ // PASS_Q   # 2
N_QC = PASS_Q // 128    # 8 q-chunks per pass
DLAG = 3            # O-matmul lag (in m-tiles) behind the exp stream

A_SC = 128.0 / np.log(2.0)      # 184.6650
DELTA = 25.0                    # common exp shift: weights = e^(s-25)
C_MAGIC = 5.0                   # schraudolph mean-bias calibration
BMAG = 16256.0 - C_MAGIC - A_SC * DELTA
SCHR_P0 = (1, 4, 7, 10, 13, 16, 19, 22, 25, 28)   # DVE exp tiles, pass 0
SCHR_P1 = (2, 7, 12, 17, 22, 27)                  # DVE exp tiles, pass 1

# xw fp16 layout: [128, XW_COLS]
W2T_OFS = 0                 # W2.T          [128, 128]
W1T_OFS = C                 # A*W1.T        [128, 128]
B2_OFS = 2 * C              # b2 column     [128, 1]
B1_OFS = 2 * C + 1          # A*b1 column   [128, 1]
XQ_OFS = 2 * C + 2          # x query half  [128, 2048]
XK_OFS = XQ_OFS + NQ        # x full        [128, 4096]
XW_COLS = XK_OFS + N

F32 = mybir.dt.float32
F16 = mybir.dt.float16
BF16 = mybir.dt.bfloat16
U16 = mybir.dt.uint16


def build_nc():
    nc = bacc.Bacc("TRN2", target_bir_lowering=False, debug=False, num_devices=8)
    xw_ext = nc.declare_dram_parameter("xw", [C, XW_COLS], F16, isOutput=False)
    xt_ext = nc.declare_dram_parameter("xt", [C, N_MT * 129], BF16, isOutput=False)
    xr_ext = nc.declare_dram_parameter("xr", [C, NQ], F32, isOutput=False)
    out_ext = nc.declare_dram_parameter("out", [C, NQ], F32, isOutput=True)

    with ExitStack() as ctx:
        tc = ctx.enter_context(tile.TileContext(nc))
        consts = ctx.enter_context(tc.tile_pool(name="consts", bufs=1))
        sb_in = ctx.enter_context(tc.tile_pool(name="sb_in", bufs=1))
        sb_kq = ctx.enter_context(tc.tile_pool(name="sb_kq", bufs=1))
        sb_e = ctx.enter_context(tc.tile_pool(name="sb_e", bufs=1))
        sb_tail = ctx.enter_context(tc.tile_pool(name="sb_tail", bufs=2))
        ps_s = ctx.enter_context(tc.tile_pool(name="ps_s", bufs=2, space="PSUM"))
        ps_o = ctx.enter_context(tc.tile_pool(name="ps_o", bufs=1, space="PSUM"))
        ps_p = ctx.enter_context(tc.tile_pool(name="ps_p", bufs=1, space="PSUM"))

        # warm the PE's HAM clock gate during the input-DMA wait; its memset
        # goes first so the warmup starts as early as possible
        wmm = consts.tile([C, QB], F16, tag="wmm")
        nc.vector.memset(wmm[:], 0.0)
        # dummy zero weights for O-chain heads
        zw = consts.tile([C, MT], F16, tag="zw")
        nc.vector.memset(zw[:], 0.0)
        for i in range(10):
            wps = ps_p.tile([C, QB], F32, tag="proj")
            nc.tensor.matmul(wps[:, 0:256], wmm[:, 0:C], wmm[:, 0:256],
                             start=True, stop=True)
        bexp = consts.tile([C, 1], F32, tag="bexp")
        nc.vector.memset(bexp[:], -DELTA)
        # warm the exp table early (ACT_TABLE_LOAD ~2.7us)
        warm = consts.tile([1, 16], F32, tag="warm")
        nc.vector.memset(warm[:], 0.0)
        warm_o = consts.tile([1, 16], F32, tag="warm_o")
        nc.scalar.activation(warm_o[:], warm[:], mybir.ActivationFunctionType.Exp)

        xw = sb_in.tile([C, XW_COLS], F16, tag="xw")
        xt1 = sb_in.tile([C, N_MT * 129], BF16, tag="xt1")
        xrT = sb_in.tile([C, NQ], F32, tag="xrT")
        # chunked input DMAs ordered so the first S-matmuls start early and
        # the first O-matmuls (xt1 head tiles) never block the in-order PE
        nc.sync.dma_start(xw[:, 0:XQ_OFS], xw_ext[:, 0:XQ_OFS])
        nc.sync.dma_start(xw[:, XQ_OFS:XQ_OFS + QB],
                          xw_ext[:, XQ_OFS:XQ_OFS + QB])
        nc.sync.dma_start(xw[:, XK_OFS:XK_OFS + QB],
                          xw_ext[:, XK_OFS:XK_OFS + QB])
        nc.sync.dma_start(xt1[:, 0:8 * 129], xt_ext[:, 0:8 * 129])
        nc.sync.dma_start(xw[:, XK_OFS + QB:XK_OFS + 2 * QB],
                          xw_ext[:, XK_OFS + QB:XK_OFS + 2 * QB])
        nc.sync.dma_start(xw[:, XQ_OFS + QB:XQ_OFS + 2 * QB],
                          xw_ext[:, XQ_OFS + QB:XQ_OFS + 2 * QB])
        nc.sync.dma_start(xw[:, XK_OFS + 2 * QB:XK_OFS + 3 * QB],
                          xw_ext[:, XK_OFS + 2 * QB:XK_OFS + 3 * QB])
        nc.sync.dma_start(xt1[:, 8 * 129:20 * 129], xt_ext[:, 8 * 129:20 * 129])
        nc.sync.dma_start(xw[:, XQ_OFS + 2 * QB:XQ_OFS + NQ],
                          xw_ext[:, XQ_OFS + 2 * QB:XQ_OFS + NQ])
        nc.sync.dma_start(xw[:, XK_OFS + 3 * QB:XK_OFS + 5 * QB],
                          xw_ext[:, XK_OFS + 3 * QB:XK_OFS + 5 * QB])
        nc.sync.dma_start(xt1[:, 20 * 129:N_MT * 129],
                          xt_ext[:, 20 * 129:N_MT * 129])
        nc.sync.dma_start(xw[:, XK_OFS + 5 * QB:XK_OFS + N],
                          xw_ext[:, XK_OFS + 5 * QB:XK_OFS + N])
        nc.sync.dma_start(xrT[:], xr_ext[:])

        # biases as fp32 [128, 2] (ACT bias wants fp32)
        bias2 = consts.tile([C, 2], F32, tag="bias2")
        nc.vector.tensor_copy(bias2[:], xw[:, B2_OFS:B1_OFS + 1])

        kt = sb_kq.tile([C, N], F16, tag="kt")       # K = W2 x + b2
        qt = sb_kq.tile([C, NQ], F16, tag="qt")      # Q' = A*(W1 x + b1)

        def proj(dst, w_ofs, b_col, x_ofs, j, on_act, cols=QB):
            ps = ps_p.tile([C, QB], F32, tag="proj")
            nc.tensor.matmul(ps[:, 0:cols], xw[:, w_ofs:w_ofs + C],
                             xw[:, x_ofs + j * QB:x_ofs + j * QB + cols],
                             start=True, stop=True)
            if on_act:
                nc.scalar.activation(
                    dst[:, j * QB:j * QB + cols], ps[:, 0:cols],
                    mybir.ActivationFunctionType.Identity,
                    bias=bias2[:, b_col:b_col + 1])
            else:
                h = cols // 2
                for s0 in range(2):
                    nc.vector.tensor_scalar(
                        out=dst[:, j * QB + s0 * h:j * QB + (s0 + 1) * h],
                        in0=ps[:, s0 * h:(s0 + 1) * h],
                        scalar1=bias2[:, b_col:b_col + 1], scalar2=None,
                        op0=mybir.AluOpType.add)

        # prologue: projections the first S-matmuls need; the rest sprinkle
        # into pass 0 so the exp stream starts early
        proj(qt, W1T_OFS, 1, XQ_OFS, 0, True)
        proj(kt, W2T_OFS, 0, XK_OFS, 0, False)
        proj(qt, W1T_OFS, 1, XQ_OFS, 1, True)
        # K-chunk c feeds S-matmuls from m-tile 4c; sprinkle at step 2(c-1)
        sprinkle = {0: (kt, W2T_OFS, 0, XK_OFS, 1), 2: (kt, W2T_OFS, 0, XK_OFS, 2),
                    4: (kt, W2T_OFS, 0, XK_OFS, 3), 6: (kt, W2T_OFS, 0, XK_OFS, 4),
                    8: (kt, W2T_OFS, 0, XK_OFS, 5), 10: (kt, W2T_OFS, 0, XK_OFS, 6),
                    12: (kt, W2T_OFS, 0, XK_OFS, 7), 14: (qt, W1T_OFS, 1, XQ_OFS, 2),
                    16: (qt, W1T_OFS, 1, XQ_OFS, 3)}

        # E staged for a whole pass in SBUF (stationary source for O)
        e_stage = sb_e.tile([C, N_MT * PASS_Q], BF16, tag="e")

        for p in range(N_PASS):
            q0 = p * PASS_Q
            schr_mts = SCHR_P0 if p == 0 else SCHR_P1
            # O psum windows: 8 chunks of [128, 129] packed 3-3-2 in 3 banks
            o_pa = ps_o.tile([C, 3 * 129], F32, tag="oa")
            o_pb = ps_o.tile([C, 3 * 129], F32, tag="ob")
            o_pc = ps_o.tile([C, 2 * 129], F32, tag="oc")

            def o_win(qc):
                t, k = ((o_pa, qc) if qc < 3 else
                        ((o_pb, qc - 3) if qc < 6 else (o_pc, qc - 6)))
                return t[:, k * 129:(k + 1) * 129]

            def do_s(mt):
                s_ps = ps_s.tile([C, PASS_Q], F32, tag="s")
                for j in range(2):
                    nc.tensor.matmul(
                        s_ps[:, j * QB:(j + 1) * QB],
                        kt[:, mt * MT:(mt + 1) * MT],
                        qt[:, q0 + j * QB:q0 + (j + 1) * QB],
                        start=True, stop=True)
                e_g = e_stage[:, mt * PASS_Q:(mt + 1) * PASS_Q]
                if mt in schr_mts:
                    nc.vector.tensor_scalar(
                        out=e_g.bitcast(U16), in0=s_ps[:],
                        scalar1=A_SC, scalar2=BMAG,
                        op0=mybir.AluOpType.mult, op1=mybir.AluOpType.add)
                else:
                    nc.scalar.activation(e_g, s_ps[:],
                                         mybir.ActivationFunctionType.Exp,
                                         bias=bexp[:, 0:1])

            def do_o(mt):
                for qc in range(N_QC):
                    nc.tensor.matmul(
                        o_win(qc),
                        e_stage[:, mt * PASS_Q + qc * 128:
                                   mt * PASS_Q + (qc + 1) * 128],
                        xt1[:, mt * 129:(mt + 1) * 129],
                        start=False, stop=(mt == N_MT - 1))

            for mt in range(N_MT + DLAG):
                if p == 0 and mt in sprinkle:
                    dst, w_ofs, b_col, x_ofs, j = sprinkle[mt]
                    proj(dst, w_ofs, b_col, x_ofs, j, False)
                if mt < N_MT:
                    do_s(mt)
                if mt == DLAG:
                    # zero-weight dummy heads open the 8 accumulation
                    # chains; emitted late so they don't stall the in-order
                    # PE queue on the previous pass's tail reads
                    for qc in range(N_QC):
                        nc.tensor.matmul(o_win(qc), zw[:], xt1[:, 0:129],
                                         start=True, stop=False)
                if mt >= DLAG:
                    do_o(mt - DLAG)

            # tail: strided 1/r per psum tile, ACT normalize-evac with
            # per-partition scale=bc, DVE residual add + relu, DMA out
            out_sb = sb_tail.tile([C, PASS_Q], F32, tag="out_sb")
            bc_all = sb_tail.tile([C, N_QC], F32, tag="bc_all")
            nc.vector.reciprocal_approx_fast(bc_all[:, 0:3], o_pa[:, 128::129])
            nc.vector.reciprocal_approx_fast(bc_all[:, 3:6], o_pb[:, 128::129])
            nc.vector.reciprocal_approx_fast(bc_all[:, 6:8], o_pc[:, 128::129])
            att_sb = sb_tail.tile([C, PASS_Q], F16, tag="att_sb")
            for qc in range(N_QC):
                if qc < 4:
                    nc.scalar.activation(
                        att_sb[:, qc * 128:(qc + 1) * 128],
                        o_win(qc)[:, 0:128],
                        mybir.ActivationFunctionType.Identity,
                        scale=bc_all[:, qc:qc + 1])
                    t2 = sb_tail.tile([C, 128], F32, tag="t2")
                    nc.vector.tensor_tensor(
                        t2[:], att_sb[:, qc * 128:(qc + 1) * 128],
                        xrT[:, q0 + qc * 128:q0 + (qc + 1) * 128],
                        op=mybir.AluOpType.add)
                else:
                    t2 = sb_tail.tile([C, 128], F32, tag="t2")
                    nc.vector.scalar_tensor_tensor(
                        out=t2[:], in0=o_win(qc)[:, 0:128],
                        scalar=bc_all[:, qc:qc + 1],
                        in1=xrT[:, q0 + qc * 128:q0 + (qc + 1) * 128],
                        op0=mybir.AluOpType.mult, op1=mybir.AluOpType.add)
                eng = nc.gpsimd if qc >= 6 else nc.vector
                eng.tensor_scalar_max(
                    out_sb[:, qc * 128:(qc + 1) * 128], t2[:], 0.0)
                if qc % 2 == 1:
                    g0 = qc - 1
                    nc.gpsimd.dma_start(
                        out_ext[:, q0 + g0 * 128:q0 + (qc + 1) * 128],
                        out_sb[:, g0 * 128:(qc + 1) * 128])

    nc.compile()
    return nc


_NC_CACHE = None


def _get_nc():
    global _NC_CACHE
    if _NC_CACHE is None:
        _NC_CACHE = build_nc()
    return _NC_CACHE


def make_in_maps(x, W1, b1, W2, b2):
    x = np.asarray(x, np.float32)
    W1 = np.asarray(W1, np.float32)
    b1 = np.asarray(b1, np.float32)
    W2 = np.asarray(W2, np.float32)
    b2 = np.asarray(b2, np.float32)
    in_maps = []
    for core in range(8):
        b, h = divmod(core, 2)
        xb = x[b]                                    # [128, 4096]
        xq = xb[:, h * NQ:(h + 1) * NQ]
        xw = np.empty((C, XW_COLS), np.float16)
        xw[:, W2T_OFS:W2T_OFS + C] = W2.T
        xw[:, W1T_OFS:W1T_OFS + C] = W1.T
        xw[:, B2_OFS] = b2
        xw[:, B1_OFS] = b1
        xw[:, XQ_OFS:XQ_OFS + NQ] = xq
        xw[:, XK_OFS:XK_OFS + N] = xb
        # xt1 tile mt: cols [mt*129, mt*129+128) = x[c, mt*128+p], col
        # mt*129+128 = ones
        xt1 = np.empty((C, N_MT * 129), ml_dtypes.bfloat16)
        xtT = xb.T.reshape(N_MT, MT, C)              # [mt, p, c]
        for mt in range(N_MT):
            xt1[:, mt * 129:mt * 129 + C] = xtT[mt]
            xt1[:, mt * 129 + C] = 1.0
        # xrT fp32: [p, qc*128 + c] = x[c, h*NQ + qc*128 + p]
        xrT = np.ascontiguousarray(
            xq.T.reshape(NQ // 128, 128, C).transpose(1, 0, 2).reshape(128, NQ)
        ).astype(np.float32)
        in_maps.append({"xw": xw, "xt": xt1, "xr": xrT})
    return in_maps


def run(x, W1, b1, W2, b2, trace=False):
    nc = _get_nc()
    in_maps = make_in_maps(x, W1, b1, W2, b2)
    last_err = None
    for _attempt in range(3):
        try:
            res = run_bass_kernel_spmd(nc, in_maps, core_ids=list(range(8)),
                                       trace=trace)
            break
        except Exception as e:  # transient NRT/device errors: retry
            last_err = e
    else:
        raise last_err
    out = np.empty((B, C, N), np.float32)
    for core in range(8):
        b, h = divmod(core, 2)
        o = res.results[core]["out"]                 # [128 p, 2048 qc*c]
        # out[c, h*NQ + qc*128 + p] = o[p, qc*128 + c]
        ob = o.reshape(128, NQ // 128, C).transpose(2, 1, 0).reshape(C, NQ)
        out[b][:, h * NQ:(h + 1) * NQ] = ob
    return out, res


def kernel(x, W1, b1, W2, b2):
    out, _ = run(x, W1, b1, W2, b2, trace=False)
    return out


# revision 8
# speedup vs baseline: 1.0502x; 1.0502x over previous
"""Trainium2 Bass kernel for nn_Attention_4063039062503.

Reference (per batch b, C=128 channels, N=4096 points):
    q = W1 @ x + b1;  k = W2 @ x + b2          # [C, N]
    s[n, m] = q[:, n] . k[:, m]                # [N, N]
    a = softmax(s, axis=m)
    out = relu(x + x @ a.T)                    # out[:, n] = x @ a[n, :]

Sharding: 8 cores, core i -> batch i//2, query half i%2 (2048 queries),
full 4096 keys local (no collectives).

Per-core plan (v2, E-stationary O-matmul):
  - exp split across engines computing IDENTICAL e^(s-25) per tile:
      ACT: exp(s - 25) -> bf16 (exact, bias-only)
      DVE: Schraudolph bits = s*(128/ln2) + BMAG -> uint16 == bf16(e^(s-25));
           the fp32->uint16 convert saturates negatives to 0 (measured), so
           no explicit clamp is needed
    (mixed-tile bias calibrated via BMAG c=5; end-to-end sim err 6.5e-3)
  - O-matmul flipped: stationary = E q-chunk [m,128], moving = [x^T | 1]
    [m,129] -> psum O'[q, 0:128] with the softmax row-sum landing free in
    col 128 (kills the DVE row-sum accumulation entirely)
  - zero-weight dummy matmuls open each O accumulation chain: hardware
    drops the first term of a start=True 129-col chain (measured), dummies
    make every real matmul start=False
  - tail per q-chunk: 1/r via reciprocal, (O*bc + x^T) on DVE, relu, DMA
    out in [q, c] orientation; host transposes back
"""
from contextlib import ExitStack

import numpy as np
import ml_dtypes

import concourse.tile as tile
from concourse import bacc, mybir
from concourse.bass_utils import run_bass_kernel_spmd

B = 4
C = 128
N = 4096            # keys per batch
NQ = 2048           # queries per core
QB = 512            # S-matmul moving block (PSUM bank)
MT = 128            # m (key) tile
N_MT = N // MT      # 32
PASS_Q = 1024       # queries per pass
N_PASS = NQ // PASS_Q   # 2
N_QC = PASS_Q // 128    # 8 q-chunks per pass
DLAG = 3            # O-matmul lag (in m-tiles) behind the exp stream

A_SC = 128.0 / np.log(2.0)      # 184.6650
DELTA = 25.0                    # common exp shift: weights = e^(s-25)
C_MAGIC = 5.0                   # schraudolph mean-bias calibration
BMAG = 16256.0 - C_MAGIC - A_SC * DELTA
SCHR_P0 = (1, 4, 7, 10, 13, 16, 19, 22, 25, 28)   # DVE exp tiles, pass 0
SCHR_P1 = (2, 7, 12, 17, 22, 27)                  # DVE exp tiles, pass 1

# xw fp16 layout: [128, XW_COLS]
W2T_OFS = 0                 # W2.T          [128, 128]
W1T_OFS = C                 # A*W1.T        [128, 128]
B2_OFS = 2 * C              # b2 column     [128, 1]
B1_OFS = 2 * C + 1          # A*b1 column   [128, 1]
XQ_OFS = 2 * C + 2          # x query half  [128, 2048]
XK_OFS = XQ_OFS + NQ        # x full        [128, 4096]
XW_COLS = XK_OFS + N

F32 = mybir.dt.float32
F16 = mybir.dt.float16
BF16 = mybir.dt.bfloat16
U16 = mybir.dt.uint16


def build_nc():
    nc = bacc.Bacc("TRN2", target_bir_lowering=False, debug=False, num_devices=8)
    xw_ext = nc.declare_dram_parameter("xw", [C, XW_COLS], F16, isOutput=False)
    xt_ext = nc.declare_dram_parameter("xt", [C, N_MT * 129], BF16, isOutput=False)
    xr_ext = nc.declare_dram_parameter("xr", [C, NQ], F32, isOutput=False)
    out_ext = nc.declare_dram_parameter("out", [C, NQ], F32, isOutput=True)

    with ExitStack() as ctx:
        tc = ctx.enter_context(tile.TileContext(nc))
        consts = ctx.enter_context(tc.tile_pool(name="consts", bufs=1))
        sb_in = ctx.enter_context(tc.tile_pool(name="sb_in", bufs=1))
        sb_kq = ctx.enter_context(tc.tile_pool(name="sb_kq", bufs=1))
        sb_e = ctx.enter_context(tc.tile_pool(name="sb_e", bufs=1))
        sb_tail = ctx.enter_context(tc.tile_pool(name="sb_tail", bufs=2))
        ps_s = ctx.enter_context(tc.tile_pool(name="ps_s", bufs=2, space="PSUM"))
        ps_o = ctx.enter_context(tc.tile_pool(name="ps_o", bufs=1, space="PSUM"))
        ps_p = ctx.enter_context(tc.tile_pool(name="ps_p", bufs=1, space="PSUM"))

        # warm the PE's HAM clock gate during the input-DMA wait; its memset
        # goes first so the warmup starts as early as possible
        wmm = consts.tile([C, QB], F16, tag="wmm")
        nc.vector.memset(wmm[:], 0.0)
        # dummy zero weights for O-chain heads
        zw = consts.tile([C, MT], F16, tag="zw")
        nc.vector.memset(zw[:], 0.0)
        for i in range(10):
            wps = ps_p.tile([C, QB], F32, tag="proj")
            nc.tensor.matmul(wps[:, 0:256], wmm[:, 0:C], wmm[:, 0:256],
                             start=True, stop=True)
        bexp = consts.tile([C, 1], F32, tag="bexp")
        nc.vector.memset(bexp[:], -DELTA)
        # warm the exp table early (ACT_TABLE_LOAD ~2.7us)
        warm = consts.tile([1, 16], F32, tag="warm")
        nc.vector.memset(warm[:], 0.0)
        warm_o = consts.tile([1, 16], F32, tag="warm_o")
        nc.scalar.activation(warm_o[:], warm[:], mybir.ActivationFunctionType.Exp)

        xw = sb_in.tile([C, XW_COLS], F16, tag="xw")
        xt1 = sb_in.tile([C, N_MT * 129], BF16, tag="xt1")
        xrT = sb_in.tile([C, NQ], F32, tag="xrT")
        # chunked input DMAs ordered so the first S-matmuls start early and
        # the first O-matmuls (xt1 head tiles) never block the in-order PE
        nc.sync.dma_start(xw[:, 0:XQ_OFS], xw_ext[:, 0:XQ_OFS])
        nc.sync.dma_start(xw[:, XQ_OFS:XQ_OFS + QB],
                          xw_ext[:, XQ_OFS:XQ_OFS + QB])
        nc.sync.dma_start(xw[:, XK_OFS:XK_OFS + QB],
                          xw_ext[:, XK_OFS:XK_OFS + QB])
        nc.sync.dma_start(xt1[:, 0:8 * 129], xt_ext[:, 0:8 * 129])
        nc.sync.dma_start(xw[:, XK_OFS + QB:XK_OFS + 2 * QB],
                          xw_ext[:, XK_OFS + QB:XK_OFS + 2 * QB])
        nc.sync.dma_start(xw[:, XQ_OFS + QB:XQ_OFS + 2 * QB],
                          xw_ext[:, XQ_OFS + QB:XQ_OFS + 2 * QB])
        nc.sync.dma_start(xw[:, XK_OFS + 2 * QB:XK_OFS + 3 * QB],
                          xw_ext[:, XK_OFS + 2 * QB:XK_OFS + 3 * QB])
        nc.sync.dma_start(xt1[:, 8 * 129:20 * 129], xt_ext[:, 8 * 129:20 * 129])
        nc.sync.dma_start(xw[:, XQ_OFS + 2 * QB:XQ_OFS + NQ],
                          xw_ext[:, XQ_OFS + 2 * QB:XQ_OFS + NQ])
        nc.sync.dma_start(xw[:, XK_OFS + 3 * QB:XK_OFS + 5 * QB],
                          xw_ext[:, XK_OFS + 3 * QB:XK_OFS + 5 * QB])
        nc.sync.dma_start(xt1[:, 20 * 129:N_MT * 129],
                          xt_ext[:, 20 * 129:N_MT * 129])
        nc.sync.dma_start(xw[:, XK_OFS + 5 * QB:XK_OFS + N],
                          xw_ext[:, XK_OFS + 5 * QB:XK_OFS + N])
        nc.sync.dma_start(xrT[:], xr_ext[:])

        # biases as fp32 [128, 2] (ACT bias wants fp32)
        bias2 = consts.tile([C, 2], F32, tag="bias2")
        nc.vector.tensor_copy(bias2[:], xw[:, B2_OFS:B1_OFS + 1])

        kt = sb_kq.tile([C, N], F16, tag="kt")       # K = W2 x + b2
        qt = sb_kq.tile([C, NQ], F16, tag="qt")      # Q' = A*(W1 x + b1)

        def proj(dst, w_ofs, b_col, x_ofs, j, on_act, cols=QB):
            ps = ps_p.tile([C, QB], F32, tag="proj")
            nc.tensor.matmul(ps[:, 0:cols], xw[:, w_ofs:w_ofs + C],
                             xw[:, x_ofs + j * QB:x_ofs + j * QB + cols],
                             start=True, stop=True)
            if on_act:
                nc.scalar.activation(
                    dst[:, j * QB:j * QB + cols], ps[:, 0:cols],
                    mybir.ActivationFunctionType.Identity,
                    bias=bias2[:, b_col:b_col + 1])
            else:
                h = cols // 2
                for s0 in range(2):
                    nc.vector.tensor_scalar(
                        out=dst[:, j * QB + s0 * h:j * QB + (s0 + 1) * h],
                        in0=ps[:, s0 * h:(s0 + 1) * h],
                        scalar1=bias2[:, b_col:b_col + 1], scalar2=None,
                        op0=mybir.AluOpType.add)

        # prologue: projections the first S-matmuls need; the rest sprinkle
        # into pass 0 so the exp stream starts early
        proj(qt, W1T_OFS, 1, XQ_OFS, 0, True)
        proj(kt, W2T_OFS, 0, XK_OFS, 0, False)
        proj(qt, W1T_OFS, 1, XQ_OFS, 1, True)
        # K-chunk c feeds S-matmuls from m-tile 4c; sprinkle at step 2(c-1)
        sprinkle = {0: (kt, W2T_OFS, 0, XK_OFS, 1), 2: (kt, W2T_OFS, 0, XK_OFS, 2),
                    4: (kt, W2T_OFS, 0, XK_OFS, 3), 6: (kt, W2T_OFS, 0, XK_OFS, 4),
                    8: (kt, W2T_OFS, 0, XK_OFS, 5), 10: (kt, W2T_OFS, 0, XK_OFS, 6),
                    12: (kt, W2T_OFS, 0, XK_OFS, 7), 14: (qt, W1T_OFS, 1, XQ_OFS, 2),
                    16: (qt, W1T_OFS, 1, XQ_OFS, 3)}

        # E staged for a whole pass in SBUF (stationary source for O)
        e_stage = sb_e.tile([C, N_MT * PASS_Q], BF16, tag="e")

        for p in range(N_PASS):
            q0 = p * PASS_Q
            schr_mts = SCHR_P0 if p == 0 else SCHR_P1
            # O psum windows: 8 chunks of [128, 129] packed 3-3-2 in 3 banks
            o_pa = ps_o.tile([C, 3 * 129], F32, tag="oa")
            o_pb = ps_o.tile([C, 3 * 129], F32, tag="ob")
            o_pc = ps_o.tile([C, 2 * 129], F32, tag="oc")

            def o_win(qc):
                t, k = ((o_pa, qc) if qc < 3 else
                        ((o_pb, qc - 3) if qc < 6 else (o_pc, qc - 6)))
                return t[:, k * 129:(k + 1) * 129]

            def do_s(mt):
                s_ps = ps_s.tile([C, PASS_Q], F32, tag="s")
                for j in range(2):
                    nc.tensor.matmul(
                        s_ps[:, j * QB:(j + 1) * QB],
                        kt[:, mt * MT:(mt + 1) * MT],
                        qt[:, q0 + j * QB:q0 + (j + 1) * QB],
                        start=True, stop=True)
                e_g = e_stage[:, mt * PASS_Q:(mt + 1) * PASS_Q]
                if mt in schr_mts:
                    nc.vector.tensor_scalar(
                        out=e_g.bitcast(U16), in0=s_ps[:],
                        scalar1=A_SC, scalar2=BMAG,
                        op0=mybir.AluOpType.mult, op1=mybir.AluOpType.add)
                else:
                    nc.scalar.activation(e_g, s_ps[:],
                                         mybir.ActivationFunctionType.Exp,
                                         bias=bexp[:, 0:1])

            def do_o(mt):
                for qc in range(N_QC):
                    nc.tensor.matmul(
                        o_win(qc),
                        e_stage[:, mt * PASS_Q + qc * 128:
                                   mt * PASS_Q + (qc + 1) * 128],
                        xt1[:, mt * 129:(mt + 1) * 129],
                        start=False, stop=(mt == N_MT - 1))

            for mt in range(N_MT + DLAG):
                if p == 0 and mt in sprinkle:
                    dst, w_ofs, b_col, x_ofs, j = sprinkle[mt]
                    proj(dst, w_ofs, b_col, x_ofs, j, False)
                if mt < N_MT:
                    do_s(mt)
                if mt == DLAG:
                    # zero-weight dummy heads open the 8 accumulation
                    # chains; emitted late so they don't stall the in-order
                    # PE queue on the previous pass's tail reads
                    for qc in range(N_QC):
                        nc.tensor.matmul(o_win(qc), zw[:], xt1[:, 0:129],
                                         start=True, stop=False)
                if mt >= DLAG:
                    do_o(mt - DLAG)

            # tail: strided 1/r per psum tile, ACT normalize-evac with
            # per-partition scale=bc, DVE residual add + relu, DMA out
            out_sb = sb_tail.tile([C, PASS_Q], F32, tag="out_sb")
            bc_all = sb_tail.tile([C, N_QC], F32, tag="bc_all")
            nc.vector.reciprocal_approx_fast(bc_all[:, 0:3], o_pa[:, 128::129])
            nc.vector.reciprocal_approx_fast(bc_all[:, 3:6], o_pb[:, 128::129])
            nc.vector.reciprocal_approx_fast(bc_all[:, 6:8], o_pc[:, 128::129])
            att_sb = sb_tail.tile([C, PASS_Q], F16, tag="att_sb")
            for qc in range(N_QC):
                if qc < 4:
                    nc.scalar.activation(
                        att_sb[:, qc * 128:(qc + 1) * 128],
                        o_win(qc)[:, 0:128],
                        mybir.ActivationFunctionType.Identity,
                        scale=bc_all[:, qc:qc + 1])
                    t2 = sb_tail.tile([C, 128], F32, tag="t2")
                    nc.vector.tensor_tensor(
                        t2[:], att_sb[:, qc * 128:(qc + 1) * 128],
                        xrT[:, q0 + qc * 128:q0 + (qc + 1) * 128],
                        op=mybir.AluOpType.add)
                else:
                    t2 = sb_tail.tile([C, 128], F32, tag="t2")
                    nc.vector.scalar_tensor_tensor(
                        out=t2[:], in0=o_win(qc)[:, 0:128],
                        scalar=bc_all[:, qc:qc + 1],
                        in1=xrT[:, q0 + qc * 128:q0 + (qc + 1) * 128],
                        op0=mybir.AluOpType.mult, op1=mybir.AluOpType.add)
                nc.vector.tensor_scalar_max(
                    out_sb[:, qc * 128:(qc + 1) * 128], t2[:], 0.0)
                if qc % 2 == 1:
                    g0 = qc - 1
                    nc.gpsimd.dma_start(
                        out_ext[:, q0 + g0 * 128:q0 + (qc + 1) * 128],
                        out_sb[:, g0 * 128:(qc + 1) * 128])

    nc.compile()
    return nc


_NC_CACHE = None


def _get_nc():
    global _NC_CACHE
    if _NC_CACHE is None:
        _NC_CACHE = build_nc()
    return _NC_CACHE


def make_in_maps(x, W1, b1, W2, b2):
    x = np.asarray(x, np.float32)
    W1 = np.asarray(W1, np.float32)
    b1 = np.asarray(b1, np.float32)
    W2 = np.asarray(W2, np.float32)
    b2 = np.asarray(b2, np.float32)
    in_maps = []
    for core in range(8):
        b, h = divmod(core, 2)
        xb = x[b]                                    # [128, 4096]
        xq = xb[:, h * NQ:(h + 1) * NQ]
        xw = np.empty((C, XW_COLS), np.float16)
        xw[:, W2T_OFS:W2T_OFS + C] = W2.T
        xw[:, W1T_OFS:W1T_OFS + C] = W1.T
        xw[:, B2_OFS] = b2
        xw[:, B1_OFS] = b1
        xw[:, XQ_OFS:XQ_OFS + NQ] = xq
        xw[:, XK_OFS:XK_OFS + N] = xb
        # xt1 tile mt: cols [mt*129, mt*129+128) = x[c, mt*128+p], col
        # mt*129+128 = ones
        xt1 = np.empty((C, N_MT * 129), ml_dtypes.bfloat16)
        xtT = xb.T.reshape(N_MT, MT, C)              # [mt, p, c]
        for mt in range(N_MT):
            xt1[:, mt * 129:mt * 129 + C] = xtT[mt]
            xt1[:, mt * 129 + C] = 1.0
        # xrT fp32: [p, qc*128 + c] = x[c, h*NQ + qc*128 + p]
        xrT = np.ascontiguousarray(
            xq.T.reshape(NQ // 128, 128, C).transpose(1, 0, 2).reshape(128, NQ)
        ).astype(np.float32)
        in_maps.append({"xw": xw, "xt": xt1, "xr": xrT})
    return in_maps


def run(x, W1, b1, W2, b2, trace=False):
    nc = _get_nc()
    in_maps = make_in_maps(x, W1, b1, W2, b2)
    last_err = None
    for _attempt in range(3):
        try:
            res = run_bass_kernel_spmd(nc, in_maps, core_ids=list(range(8)),
                                       trace=trace)
            break
        except Exception as e:  # transient NRT/device errors: retry
            last_err = e
    else:
        raise last_err
    out = np.empty((B, C, N), np.float32)
    for core in range(8):
        b, h = divmod(core, 2)
        o = res.results[core]["out"]                 # [128 p, 2048 qc*c]
        # out[c, h*NQ + qc*128 + p] = o[p, qc*128 + c]
        ob = o.reshape(128, NQ // 128, C).transpose(2, 1, 0).reshape(C, NQ)
        out[b][:, h * NQ:(h + 1) * NQ] = ob
    return out, res


def kernel(x, W1, b1, W2, b2):
    out, _ = run(x, W1, b1, W2, b2, trace=False)
    return out
